# revision 2
# baseline (speedup 1.0000x reference)
"""Multi-head attention (nn_MultiHeadAttention) Trainium2 Bass kernel.

Sharding: 8 cores = 4 batches x 2 head-groups (8 heads each).
Per core: project k,q (head-group rows) and vT, S = k^T q per head with the
key mask folded into the exp bias, colsum via ones-matmul, normalize with
qmask/colsum factor (broadcast across partitions), O = vT^T @ P, then the
w_out partial projection. Host adds x + the two head-group partials and
assembles att_flat.

Matmuls run as float32r (reduced-precision fp32, 4x faster than fp32 on PE).
"""
import sys
sys.path.insert(0, "/opt/trn_rl_repo")

import numpy as np

B, C, T, H = 4, 1024, 1024, 16
D = C // H            # 64 head dim
HPC = 8               # heads per core
NCORES = 8

_CACHE = {}


def _build_program():
    import concourse.bass as bass  # noqa: F401
    import concourse.mybir as mybir
    import concourse.tile as tile
    from concourse import bacc

    f32 = mybir.dt.float32
    f32r = mybir.dt.float32r

    nc = bacc.Bacc("TRN2", target_bir_lowering=False, debug=False,
                   num_devices=NCORES)

    x_d = nc.dram_tensor("x", [C, T], f32r, kind="ExternalInput").ap()
    wk_d = nc.dram_tensor("wk", [C, 512], f32r, kind="ExternalInput").ap()
    wq_d = nc.dram_tensor("wq", [C, 512], f32r, kind="ExternalInput").ap()
    wv_d = nc.dram_tensor("wv", [C, 512], f32r, kind="ExternalInput").ap()
    wo_d = nc.dram_tensor("wo", [512, C], f32r, kind="ExternalInput").ap()
    kmask_d = nc.dram_tensor("kmask", [128, 8], f32, kind="ExternalInput").ap()
    qmask_d = nc.dram_tensor("qmask", [1, T], f32r, kind="ExternalInput").ap()
    ones_d = nc.dram_tensor("ones", [128, 1], f32r, kind="ExternalInput").ap()

    att_d = nc.dram_tensor("att", [HPC, T, T], f32r, kind="ExternalOutput").ap()
    out_d = nc.dram_tensor("out", [C, T], f32r, kind="ExternalOutput").ap()

    Exp = mybir.ActivationFunctionType.Exp

    with tile.TileContext(nc) as tc:
        with (
            tc.tile_pool(name="persist", bufs=1) as persist,
            tc.tile_pool(name="evac", bufs=4) as evac_pool,
        ):
            k_sb = persist.tile([128, 4, T], f32r)
            q_sb = persist.tile([128, 4, T], f32r)
            vT_sb = persist.tile([128, 8, 512], f32r)
            o_all = persist.tile([128, 4, T], f32r)
            wo_sb = persist.tile([128, 4, T], f32r)
            kmask_sb = persist.tile([128, 8], f32)
            qmask_sb = persist.tile([1, T], f32r)
            ones_sb = persist.tile([128, 1], f32r)
            stage_sb = persist.tile([64, T], f32r)

            nc.sync.dma_start(out=kmask_sb[:], in_=kmask_d[:])
            nc.sync.dma_start(out=qmask_sb[:], in_=qmask_d[:])
            nc.sync.dma_start(out=ones_sb[:], in_=ones_d[:])
            nc.sync.dma_start(
                out=wo_sb[:], in_=wo_d.rearrange("(a p) c -> p a c", p=128))

            # ---- projections ----
            with (
                tc.tile_pool(name="projin", bufs=1) as projin,
                tc.tile_pool(name="proj_ps", bufs=4, space="PSUM") as proj_ps,
            ):
                x_sb = projin.tile([128, 8, T], f32r)
                wk_sb = projin.tile([128, 8, 512], f32r)
                wq_sb = projin.tile([128, 8, 512], f32r)
                wv_sb = projin.tile([128, 8, 512], f32r)
                nc.sync.dma_start(
                    out=x_sb[:], in_=x_d.rearrange("(a p) t -> p a t", p=128))
                nc.sync.dma_start(
                    out=wk_sb[:], in_=wk_d.rearrange("(a p) m -> p a m", p=128))
                nc.sync.dma_start(
                    out=wq_sb[:], in_=wq_d.rearrange("(a p) m -> p a m", p=128))
                nc.sync.dma_start(
                    out=wv_sb[:], in_=wv_d.rearrange("(a p) m -> p a m", p=128))

                # k, q projections: out rows m-tile (128 rows = head pair)
                for w_sb, dst in ((wk_sb, k_sb), (wq_sb, q_sb)):
                    for m in range(4):
                        for n in range(2):
                            ps = proj_ps.tile([128, 512], f32, tag="pp")
                            for a in range(8):
                                nc.tensor.matmul(
                                    ps[:],
                                    w_sb[:, a, 128 * m:128 * m + 128],
                                    x_sb[:, a, 512 * n:512 * n + 512],
                                    start=(a == 0), stop=(a == 7),
                                )
                            nc.scalar.copy(
                                out=dst[:, m, 512 * n:512 * n + 512], in_=ps[:])
                # vT projection: out [t-tile, 512 cv]
                for m in range(8):
                    ps = proj_ps.tile([128, 512], f32, tag="pp")
                    for a in range(8):
                        nc.tensor.matmul(
                            ps[:],
                            x_sb[:, a, 128 * m:128 * m + 128],
                            wv_sb[:, a, :],
                            start=(a == 0), stop=(a == 7),
                        )
                    nc.vector.tensor_copy(vT_sb[:, m, :], ps[:])

            # ---- attention per head ----
            with (
                tc.tile_pool(name="exps", bufs=12) as exps_pool,
                tc.tile_pool(name="rb", bufs=2) as rb_pool,
                tc.tile_pool(name="rc", bufs=2) as rc_pool,
                tc.tile_pool(name="s_ps", bufs=2, space="PSUM") as s_ps,
                tc.tile_pool(name="cs_ps", bufs=1, space="PSUM") as cs_ps,
                tc.tile_pool(name="o_ps", bufs=1, space="PSUM") as o_ps,
            ):
                for h in range(HPC):
                    hb = 64 * (h % 2)
                    hp = h // 2
                    cs = cs_ps.tile([1, T], f32, tag="cs")
                    exs = []
                    for it in range(8):
                        sp = s_ps.tile([128, T], f32, tag="s")
                        for jn in range(2):
                            nc.tensor.matmul(
                                sp[:, 512 * jn:512 * jn + 512],
                                k_sb[hb:hb + 64, hp, 128 * it:128 * it + 128],
                                q_sb[hb:hb + 64, hp, 512 * jn:512 * jn + 512],
                                start=True, stop=True,
                            )
                        ex = exps_pool.tile([128, T], f32r, tag="exps")
                        nc.scalar.activation(
                            out=ex[:], in_=sp[:], func=Exp,
                            bias=kmask_sb[:, it:it + 1], scale=1.0)
                        for jn in range(2):
                            nc.tensor.matmul(
                                cs[0:1, 512 * jn:512 * jn + 512],
                                ones_sb[:],
                                ex[:, 512 * jn:512 * jn + 512],
                                start=(it == 0), stop=(it == 7),
                            )
                        exs.append(ex)
                    rc = rc_pool.tile([1, T], f32r, tag="rc")
                    with nc.allow_low_precision(reason="f32r recip is plenty for att normalize"):
                        nc.vector.reciprocal(rc[:], cs[0:1, :])
                    nc.vector.tensor_mul(rc[:], rc[:], qmask_sb[:])
                    rb = rb_pool.tile([128, T], f32r, tag="rb")
                    nc.gpsimd.partition_broadcast(rb[:], rc[0:1, :])

                    op = o_ps.tile([64, T], f32, tag="op")
                    for it in range(8):
                        ex = exs[it]
                        nc.vector.tensor_mul(ex[:], ex[:], rb[:])
                        nc.sync.dma_start(
                            out=att_d[h, 128 * it:128 * it + 128, :], in_=ex[:])
                        for jn in range(2):
                            nc.tensor.matmul(
                                op[:, 512 * jn:512 * jn + 512],
                                vT_sb[:, it, 64 * h:64 * h + 64],
                                ex[:, 512 * jn:512 * jn + 512],
                                start=(it == 0), stop=(it == 7),
                            )
                    if h % 2 == 0:
                        nc.scalar.copy(out=o_all[0:64, hp, :], in_=op[:])
                    else:
                        nc.scalar.copy(out=stage_sb[:], in_=op[:])
                        nc.sync.dma_start(
                            out=o_all[64:128, hp, :], in_=stage_sb[:])

            # ---- output projection (partial: host adds x and the peer core) ----
            with (
                tc.tile_pool(name="out_sb", bufs=3) as out_pool,
                tc.tile_pool(name="out_ps", bufs=2, space="PSUM") as out_ps,
            ):
                for m in range(8):
                    ot = out_pool.tile([128, T], f32r, tag="ot")
                    for n in range(2):
                        ps = out_ps.tile([128, 512], f32, tag="ops")
                        for a in range(4):
                            nc.tensor.matmul(
                                ps[:],
                                wo_sb[:, a, 128 * m:128 * m + 128],
                                o_all[:, a, 512 * n:512 * n + 512],
                                start=(a == 0), stop=(a == 3),
                            )
                        nc.vector.tensor_copy(
                            ot[:, 512 * n:512 * n + 512], ps[:])
                    nc.sync.dma_start(
                        out=out_d[128 * m:128 * m + 128, :], in_=ot[:])

    nc.compile()
    return nc


def _get_program():
    if "nc" not in _CACHE:
        _CACHE["nc"] = _build_program()
    return _CACHE["nc"]


def make_in_maps(x, mask, w_kvq, w_out):
    """Build the 8 per-core input maps (core c = batch c//2, head-group c%2)."""
    x = np.asarray(x, dtype=np.float32)
    mask = np.asarray(mask)
    w_kvq = np.asarray(w_kvq, dtype=np.float32)
    w_out = np.asarray(w_out, dtype=np.float32)

    w_k, w_v, w_q = w_kvq[0:C], w_kvq[C:2 * C], w_kvq[2 * C:3 * C]
    ones = np.ones((128, 1), dtype=np.float32)
    in_maps = []
    for c in range(NCORES):
        bb, g = c // 2, c % 2
        rows = slice(512 * g, 512 * (g + 1))
        madd = np.where(mask[bb], np.float32(-1e30), np.float32(0.0)).astype(np.float32)
        in_maps.append({
            "x": np.ascontiguousarray(x[bb]),
            "wk": np.ascontiguousarray((w_k[rows] / 8.0).T.astype(np.float32)),
            "wq": np.ascontiguousarray(w_q[rows].T),
            "wv": np.ascontiguousarray(w_v[rows].T),
            "wo": np.ascontiguousarray(w_out[:, rows].T),
            "kmask": np.ascontiguousarray(madd.reshape(8, 128).T),
            "qmask": (~mask[bb]).astype(np.float32).reshape(1, T),
            "ones": ones,
        })
    return in_maps


def kernel(x, mask, w_kvq, w_out, trace=False):
    from concourse.bass_utils import run_bass_kernel_spmd

    nc = _get_program()
    in_maps = make_in_maps(x, mask, w_kvq, w_out)
    res = run_bass_kernel_spmd(nc, in_maps, core_ids=list(range(NCORES)),
                               trace=trace)

    out_full = np.empty((B, C, T), dtype=np.float32)
    att_flat = np.empty((H * B, T, T), dtype=np.float32)
    att_view = att_flat.reshape(H, B, T, T)
    x = np.asarray(x, dtype=np.float32)
    for c in range(NCORES):
        bb, g = c // 2, c % 2
        att_view[HPC * g:HPC * (g + 1), bb] = res.results[c]["att"]
        if g == 0:
            out_full[bb] = x[bb] + res.results[c]["out"]
        else:
            out_full[bb] += res.results[c]["out"]
    if trace:
        _CACHE["last_exec_time_ns"] = res.exec_time_ns
    return out_full, att_flat
